# revision 1
# baseline (speedup 1.0000x reference)
"""Trainium2 Bass kernel for nn_Attention_53455162966555.

Multi-head attention block: B=8, N=1024, DIM=1024, H=16 heads, hd=64.
Sharding: data-parallel over batch — core b computes x[b] with full weights
on NeuronCore b; no collectives. Every matmul operand is float32r (~tf32
precision at full PE speed for moving dim >= 256): measured rel err ~4e-4
vs ~3e-3 for the bf16-P/V/O/W variant (kept as kernel_bf16pv.py, ~17%% faster).

Dataflow per core: x^T via PE transposes; q^T/k^T f-tiles (2 heads stacked per
128-partition tile -> QK tile_position row-packing, K=64 pairs concurrent),
with the next pair's projection software-pipelined into the current pair's
exp window over a single shared 1-bank PSUM tag;
V in [n,d] with an appended ones-column; S^T = K.Q^T per k-tile; exp on
ScalarE straight from PSUM with the 1/8 scale fused and no max subtraction
(scores ~N(0,1)); O'^T accumulation row 64 = softmax denominators; normalize
= reciprocal -> GpSimd partition_broadcast -> multiply (factors stay fp32 —
only matmul operands need f32r); w_proj
rows permuted c=d*16+h -> c'=h*64+d by strided DMA to undo the reference's
[B,N,hd,H] output interleave; final projection on device, with the bias
(zero for this model) added exactly on the host inside kernel().

Key differences vs the bf16 variant:
  - V', expS, OT, w_proj', ones, bias all float32r (PV + proj matmuls f32r)
  - PV accumulation interleaved per kt (expS tiles are per-(head, kt) [128, N]
    instead of per-head [128, NT, N] bf16 sets — saves SBUF)
  - O' accumulators are [65, N] PSUM tiles (2 banks); softmax normalization is
    reciprocal -> GpSimd partition_broadcast -> multiply, all in fp32
  - w_proj' streamed as two f32r chunks instead of resident bf16
  - b_proj applied host-side (conditional, exact) — removes 16 K=1 bias
    matmuls from the serial projection tail
"""

import numpy as np

import concourse.bass as bass
import concourse.mybir as mybir
import concourse.tile as tile
from concourse import bacc
from concourse.masks import make_identity

P = 128
DIM = 1024
H = 16
HD = 64
F3 = 3 * DIM
CS = DIM // P
SCALE = HD ** -0.5

FP32 = mybir.dt.float32
FP32R = mybir.dt.float32r
BF16 = mybir.dt.bfloat16
Exp = mybir.ActivationFunctionType.Exp


def build_nc(N=1024):
    NT = N // P
    QC = min(512, N)
    NQ = N // QC

    nc = bacc.Bacc(None, target_bir_lowering=False)
    with tile.TileContext(nc) as tc:
        with tc.tile_pool(name="dram", bufs=1, space="DRAM") as dram:
            x_d = dram.tile([N, DIM], FP32, kind="ExternalInput")
            wqkv_d = dram.tile([DIM, F3], FP32, kind="ExternalInput")
            wproj_d = dram.tile([DIM, DIM], FP32, kind="ExternalInput")
            bproj_d = dram.tile([1, DIM], FP32, kind="ExternalInput")
            y_d = dram.tile([N, DIM], FP32, kind="ExternalOutput")
            _build_core(nc, tc, x_d, wqkv_d, wproj_d, bproj_d, y_d, N, NT, QC, NQ)
    nc.compile()
    names = dict(x=x_d.name, wqkv=wqkv_d.name, wproj=wproj_d.name,
                 bproj=bproj_d.name, y=y_d.name)
    return nc, names


def _build_core(nc, tc, x_d, wqkv_d, wproj_d, bproj_d, y_d, N, NT, QC, NQ):
    x_r = x_d[:].rearrange("(nt p) c -> p nt c", p=P)
    wqkv_r = wqkv_d[:].rearrange("(cs p) f -> p cs f", p=P)
    y_r = y_d[:].rearrange("(nt p) f -> p nt f", p=P)
    wproj_perm = wproj_d[:].rearrange("(d h2 two) f -> two d h2 f", h2=CS, two=2)

    with (
        tc.tile_pool(name="consts", bufs=1) as consts,
        tc.tile_pool(name="persist", bufs=1) as persist,
        tc.tile_pool(name="xs", bufs=2) as xs_pool,
        tc.tile_pool(name="wqs", bufs=2) as wqs_pool,
        tc.tile_pool(name="wqr", bufs=2) as wqr_pool,
        tc.tile_pool(name="stage", bufs=1) as stage_pool,
        tc.tile_pool(name="wr512", bufs=2) as wr512_pool,
        tc.tile_pool(name="qkt", bufs=2) as qkt_pool,
        tc.tile_pool(name="expst", bufs=3) as expst_pool,
        tc.tile_pool(name="recip", bufs=1) as recip_pool,
        tc.tile_pool(name="psum", bufs=1, space="PSUM") as psum,
    ):
        ident = consts.tile([P, P], FP32)
        make_identity(nc, ident[:])

        xT = persist.tile([P, CS, N], FP32R)
        V_sb = persist.tile([P, NT, H, HD + 1], FP32R)
        OT = persist.tile([P, CS, N], FP32R)
        vones_f = consts.tile([P, NT, H, 1], FP32)
        nc.vector.memset(vones_f[:], 1.0)
        nc.vector.tensor_copy(V_sb[:, :, :, HD:HD + 1], vones_f[:])

        # ---- x -> x^T --------------------------------------------------------
        for nt in range(NT):
            x_sb = xs_pool.tile([P, DIM], FP32, tag="xs")
            nc.sync.dma_start(x_sb[:], x_r[:, nt, :])
            for half in range(2):
                pt = psum.tile([P, 512], FP32, tag="u", bufs=4,
                               name=f"pt_{nt}_{half}")
                for j in range(4):
                    ct = half * 4 + j
                    nc.tensor.transpose(
                        pt[:, j * P:(j + 1) * P],
                        x_sb[:, ct * P:(ct + 1) * P],
                        ident[:],
                    )
                nc.scalar.copy(
                    xT[:, half * 4:(half + 1) * 4, nt * P:(nt + 1) * P],
                    pt[:, :].rearrange("p (cs n) -> p cs n", n=P),
                )

        # ---- V' = x @ Wv (+ones col) ----------------------------------------
        for fc in range(2):
            wv_s = stage_pool.tile([P, CS, 512], FP32, tag="stage")
            nc.sync.dma_start(
                wv_s[:],
                wqkv_r[:, :, 2 * DIM + fc * 512:2 * DIM + (fc + 1) * 512])
            wv_r = wr512_pool.tile([P, CS, 512], FP32R, tag="wr512")
            nc.vector.tensor_copy(wv_r[:], wv_s[:])
            for nt in range(NT):
                pv = psum.tile([P, 512], FP32, tag="oacc", bufs=2)
                for cs in range(CS):
                    nc.tensor.matmul(
                        pv[:], xT[:, cs, nt * P:(nt + 1) * P], wv_r[:, cs, :],
                        start=(cs == 0), stop=(cs == CS - 1),
                    )
                nc.vector.tensor_copy(
                    V_sb[:, nt, fc * 8:(fc + 1) * 8, 0:HD],
                    pv[:, :].rearrange("p (h d) -> p h d", d=HD),
                )

        # ---- q/k projection + attention per head-pair -----------------------
        def emit_qk_proj(hp):
            qk_t = qkt_pool.tile([P, 2, N], FP32R, tag="qkt",
                                 name=f"qk_t_{hp}")
            for qi, ft in enumerate((hp, CS + hp)):
                wq_s = wqs_pool.tile([P, CS, P], FP32, tag="wqs",
                                     name=f"wq_s_{hp}_{qi}")
                nc.sync.dma_start(wq_s[:], wqkv_r[:, :, ft * P:(ft + 1) * P])
                wq_r = wqr_pool.tile([P, CS, P], FP32R, tag="wqr",
                                     name=f"wq_r_{hp}_{qi}")
                nc.vector.tensor_copy(wq_r[:], wq_s[:])
                for qc in range(NQ):
                    pqk = psum.tile([P, QC], FP32, tag="u", bufs=4,
                                    name=f"pqk_{hp}_{qi}_{qc}")
                    for cs in range(CS):
                        nc.tensor.matmul(
                            pqk[:],
                            wq_r[:, cs, :],
                            xT[:, cs, qc * QC:(qc + 1) * QC],
                            start=(cs == 0), stop=(cs == CS - 1),
                        )
                    nc.vector.tensor_copy(
                        qk_t[:, qi, qc * QC:(qc + 1) * QC], pqk[:])
            return qk_t

        qk_next = emit_qk_proj(0)
        for hp in range(CS):
            qk_t = qk_next

            po_list = (0, HD)
            pacc = [psum.tile([HD + 1, N], FP32, tag="oacc", bufs=2,
                              name=f"pacc_{hp}_{hi}")
                    for hi in range(2)]
            for kt in range(NT):
                for hi, po in enumerate(po_list):
                    h = 2 * hp + hi
                    lhsT = qk_t[po:po + HD, 1, kt * P:(kt + 1) * P]
                    est = expst_pool.tile([P, N], FP32R, tag="expst",
                                          name=f"est_{hp}_{kt}_{hi}")
                    for qc in range(NQ):
                        ps = psum.tile([P, QC], FP32, tag="u", bufs=4,
                                       name=f"ps_{hp}_{kt}_{hi}_{qc}")
                        nc.tensor.matmul(
                            ps[:],
                            lhsT,
                            qk_t[po:po + HD, 0, qc * QC:(qc + 1) * QC],
                            start=True, stop=True,
                            tile_position=(po, 0),
                        )
                        nc.scalar.activation(
                            est[:, qc * QC:(qc + 1) * QC], ps[:],
                            Exp, scale=SCALE)
                        nc.tensor.matmul(
                            pacc[hi][:, qc * QC:(qc + 1) * QC],
                            V_sb[:, kt, h, :],
                            est[:, qc * QC:(qc + 1) * QC],
                            start=(kt == 0), stop=(kt == NT - 1),
                            skip_group_check=True,
                        )
            if hp + 1 < CS:
                qk_next = emit_qk_proj(hp + 1)
            # normalize: reciprocal (DVE) -> partition broadcast (GpSimd,
            # otherwise idle) -> multiply (DVE, one PSUM input). The factors
            # stay fp32 — only matmul operands need f32r; the multiply's
            # output rounds to f32r when writing OT.
            for hi, po in enumerate(po_list):
                r32 = recip_pool.tile([1, N], FP32, tag="recip32",
                                      name=f"r32_{hp}_{hi}")
                nc.vector.reciprocal(r32[:], pacc[hi][HD:HD + 1, :])
                rb = expst_pool.tile([HD, N], FP32, tag="expst",
                                     name=f"rb_{hp}_{hi}")
                nc.gpsimd.partition_broadcast(rb[:], r32[:])
                nc.vector.tensor_mul(
                    OT[po:po + HD, hp, :], pacc[hi][0:HD, :], rb[:],
                )

        # ---- y = out' @ w_proj' + b -----------------------------------------
        wp_chunks = []
        for fc in range(2):
            wp_s = stage_pool.tile([P, CS, 512], FP32, tag="stage",
                                   name=f"wp_s_{fc}")
            for half in range(2):
                nc.sync.dma_start(
                    wp_s[half * HD:(half + 1) * HD, :, :],
                    wproj_perm[half, :, :, fc * 512:(fc + 1) * 512],
                )
            wp_r = wr512_pool.tile([P, CS, 512], FP32R, tag="wr512",
                                   name=f"wp_r_{fc}")
            nc.vector.tensor_copy(wp_r[:], wp_s[:])
            wp_chunks.append(wp_r)
        for nt in range(NT):
            py_c = [psum.tile([P, 512], FP32, tag="u", bufs=4,
                              name=f"py_{nt}_{fc}")
                    for fc in range(2)]
            for cs in range(CS):
                lhsT = OT[:, cs, nt * P:(nt + 1) * P]
                for fc in range(2):
                    nc.tensor.matmul(
                        py_c[fc][:],
                        lhsT, wp_chunks[fc][:, cs, :],
                        start=(cs == 0), stop=(cs == CS - 1),
                    )
            y_sb = xs_pool.tile([P, DIM], FP32, tag="xs",
                                 name=f"y_sb_{nt}")
            for fc in range(2):
                nc.vector.tensor_copy(y_sb[:, fc * 512:(fc + 1) * 512],
                                      py_c[fc][:])
            nc.sync.dma_start(y_r[:, nt, :], y_sb[:])


_CACHE = {}


def _get_nc(N=1024):
    if N not in _CACHE:
        _CACHE[N] = build_nc(N)
    return _CACHE[N]


def kernel(x, w_qkv, w_proj, b_proj):
    """Full inputs in, full output out. Shards batch across 8 cores."""
    from concourse.bass_utils import run_bass_kernel_spmd

    B, N, C = x.shape
    assert (B, C) == (8, DIM)
    nc, nm = _get_nc(N)
    x = np.ascontiguousarray(np.asarray(x, dtype=np.float32))
    w_qkv_np = np.ascontiguousarray(np.asarray(w_qkv, dtype=np.float32))
    w_proj_np = np.ascontiguousarray(np.asarray(w_proj, dtype=np.float32))
    b_proj_np = np.ascontiguousarray(
        np.asarray(b_proj, dtype=np.float32).reshape(1, DIM))
    in_maps = [
        {nm["x"]: x[b], nm["wqkv"]: w_qkv_np, nm["wproj"]: w_proj_np,
         nm["bproj"]: b_proj_np}
        for b in range(B)
    ]
    res = run_bass_kernel_spmd(nc, in_maps, core_ids=list(range(8)))
    y = np.stack([res.results[b][nm["y"]] for b in range(B)], axis=0)
    if np.any(b_proj_np):
        # exact host-side bias add; no-op for the zero bias this model ships
        y = (y + b_proj_np.reshape(1, 1, DIM)).astype(np.float32)
    return y

